# revision 20
# baseline (speedup 1.0000x reference)
"""Trainium2 Bass kernel for the 3-layer MLP encode/decode forward pass.

Computation (B = 65536):
    d_i = pinv(W_i)                       (host, negligible)
    h = lrelu(x @ W1.T)                   [B, 128]
    h = lrelu(h @ W2.T)                   [B, 64]
    h = h @ W3.T                          [B, 16]
    h = lrelu(h @ d3.T)                   [B, 64]   (folded: lrelu((d3@W3) @ h2))
    h = lrelu(h @ d2.T)                   [B, 128]
    out = h @ d1.T                        [B, 784]

Sharding: pure data-parallel — 8 cores x 8192 batch rows each; the tiny
weights (and host-side pinv) are replicated.

Per-core layout: activations are kept feature-major ([feat, batch]) so
TensorE contracts over features.  x is transposed on-chip via PE
transpose-mode in float32r (full 2.4 GHz rate; requires x/ident DECLARED
float32r — the BIR verifier rejects bitcasts of f32 DMA data as "not
rounded to FP32r").  The final layer swaps operand roles (stationary =
activation tile, moving = d1.T) so the output lands batch-major in PSUM —
no output transpose.  Matmuls run as float32r (full PE rate at
moving-N >= 256, ~tf32 rounding).  Output is written bf16 (halves output
HBM traffic; rel err ~1.7e-3 vs the 2e-2 gate) and upcast on host.

Schedule: PSUM banks split 3/3/1 (transpose staging / L1-L4 / L5) — psMM
depth is the binding pipeline resource.  Output DMAs dispatch from the
otherwise-idle Pool (gpsimd/SWDGE) queue so input prefetch on the SP
HWDGE queue is never blocked behind output sem-waits.  One L5 copy per
tile runs on DVE to unload ACT.

DMA: one 3.2MB transfer per 1024-row group in ([128, 8*784] f32, 8 batch
rows per partition), 1.6MB bf16 out — the in-group batch permutation
cancels between input transposes and output writeback.
"""

import numpy as np

B = 65536
N_CORES = 8
B_LOC = B // N_CORES  # 8192
D0, D1, D2, D3 = 784, 128, 64, 16
KCH = 112          # 784 = 7 * 112 contraction chunks for layer 1
NKC = D0 // KCH    # 7
TILE = 512         # moving free dim per matmul (one fp32 PSUM bank)
SUB = 128          # batch sub-tile (partition dim of x / out tiles)
NSUB = TILE // SUB  # 4
HALF = D0 // 2     # 392


def _build_nc(b_loc=B_LOC, mm_dt_name="float32r", last_dt_name="float32r",
              act_name="Lrelu", repeat=1, r_xpose=False, split_ocopy=False,
              bf16_out=False, xt_on_act=False, xin_bufs=4, outp_bufs=4,
              xtp_bufs=14, acts_bufs=2, out_dma_eng="sync", in_dma_eng="sync",
              alloc_mode="stack", staggered=False, no_l5=False,
              split_in=False, ps_rebal=False, hi_in=False,
              psT_bufs=None, psMM_bufs=None, psO_bufs=None,
              l5_single_bank=False, ocopy_dve=(), l5_bf16_psum=False,
              dma_group=1):
    import contextlib
    import concourse.tile as tile
    from concourse import bacc, mybir

    mm_dt = getattr(mybir.dt, mm_dt_name)
    last_dt = getattr(mybir.dt, last_dt_name)
    f32 = mybir.dt.float32
    LRELU = getattr(mybir.ActivationFunctionType, act_name)
    COPY = mybir.ActivationFunctionType.Copy

    nc = bacc.Bacc(trn_type="TRN2", target_bir_lowering=False, debug=False,
                   num_devices=N_CORES)

    xr_dt = mybir.dt.float32r if r_xpose else f32
    x = nc.declare_dram_parameter("x", [b_loc, D0], xr_dt, isOutput=False).ap()
    w1t = nc.declare_dram_parameter("w1t", [D0, D1], mm_dt, isOutput=False).ap()
    w2t = nc.declare_dram_parameter("w2t", [D1, D2], mm_dt, isOutput=False).ap()
    m3t = nc.declare_dram_parameter("m3t", [D2, D2], mm_dt, isOutput=False).ap()
    d2t = nc.declare_dram_parameter("d2t", [D2, D1], mm_dt, isOutput=False).ap()
    d1t = nc.declare_dram_parameter("d1t", [D1, D0], last_dt, isOutput=False).ap()
    ident = nc.declare_dram_parameter("ident", [SUB, SUB], xr_dt, isOutput=False).ap()
    out_dt = mybir.dt.bfloat16 if bf16_out else f32
    out = nc.declare_dram_parameter("out", [b_loc, D0], out_dt, isOutput=True).ap()

    n_tiles = b_loc // TILE
    G = dma_group
    n_groups = n_tiles // G
    GS = NSUB * G  # batch rows per partition in one DMA group
    # row = group*(512*G) + p*(4*G) + s  (4*G rows per partition -> one
    # 1.6*G MB DMA per group; the in-group batch permutation cancels
    # between input load and output writeback)
    x_r = x.rearrange("(n p s) f -> n p (s f)", p=SUB, s=GS)
    out_r = out.rearrange("(n p s) f -> n p (s f)", p=SUB, s=GS)

    with tile.TileContext(nc, num_cores=N_CORES, pool_alloc_mode=alloc_mode) as tc:
        with (
            tc.tile_pool(name="consts", bufs=1) as consts,
            tc.tile_pool(name="xin", bufs=xin_bufs) as xin,
            tc.tile_pool(name="xtp", bufs=xtp_bufs) as xtp,
            tc.tile_pool(name="acts", bufs=acts_bufs) as acts,
            tc.tile_pool(name="outp", bufs=outp_bufs) as outp,
            tc.tile_pool(name="psT", bufs=psT_bufs or (3 if ps_rebal else 2),
                         space="PSUM") as psT,
            tc.tile_pool(name="psMM", bufs=psMM_bufs or (3 if ps_rebal else 2),
                         space="PSUM") as psMM,
            tc.tile_pool(name="psO", bufs=psO_bufs or (1 if ps_rebal else 2),
                         space="PSUM") as psO,
        ):
            # --- constants ---
            w1t_sb = consts.tile([KCH, NKC, D1], mm_dt)
            nc.sync.dma_start(out=w1t_sb, in_=w1t.rearrange("(c p) m -> p c m", p=KCH))
            w2t_sb = consts.tile([D1, D2], mm_dt)
            nc.sync.dma_start(out=w2t_sb, in_=w2t)
            m3t_sb = consts.tile([D2, D2], mm_dt)
            nc.sync.dma_start(out=m3t_sb, in_=m3t)
            d2t_sb = consts.tile([D2, D1], mm_dt)
            nc.sync.dma_start(out=d2t_sb, in_=d2t)
            d1t_sb = consts.tile([D1, D0], last_dt)
            nc.sync.dma_start(out=d1t_sb, in_=d1t)
            id_sb = consts.tile([SUB, SUB], xr_dt)
            nc.sync.dma_start(out=id_sb, in_=ident)
            id_r = id_sb

            rep_ctx = (tc.For_i(0, repeat, 1, staggered_reset=staggered)
                       if repeat > 1 else contextlib.nullcontext())
            with rep_ctx:
              for g in range(n_groups):
               # --- load 512*G rows in one DMA: [128, 4*G, 784] ---
               x_sb = xin.tile([SUB, GS, D0], xr_dt, tag="x")
               if hi_in:
                   with tc.high_priority():
                       nc.sync.dma_start(out=x_sb, in_=x_r[g])
               elif split_in:
                   xr3 = x_r[g].rearrange("p (s f) -> p s f", s=GS)
                   h = GS // 2
                   nc.sync.dma_start(out=x_sb[:, 0:h, :], in_=xr3[:, 0:h, :])
                   nc.sync.dma_start(out=x_sb[:, h:GS, :], in_=xr3[:, h:GS, :])
               elif in_dma_eng == "alt":
                   (nc.sync if g % 2 == 0 else nc.scalar).dma_start(
                       out=x_sb, in_=x_r[g])
               else:
                   getattr(nc, in_dma_eng).dma_start(out=x_sb, in_=x_r[g])
               o_sb = outp.tile([SUB, GS, D0], out_dt, tag="o")
               for t2 in range(G):
                s0 = t2 * NSUB

                # --- PE-transpose to feature-major: 7 chunks of [112, 512] ---
                xt_sb = []
                for c in range(NKC):
                    tp = psT.tile([KCH, TILE], f32, tag="psT")
                    for s in range(NSUB):
                        if r_xpose:
                            nc.tensor.transpose(
                                out=tp[:, s * SUB:(s + 1) * SUB]
                                    .bitcast(mybir.dt.float32r),
                                in_=x_sb[:, s0 + s, c * KCH:(c + 1) * KCH],
                                identity=id_r,
                            )
                        else:
                            nc.tensor.transpose(
                                out=tp[:, s * SUB:(s + 1) * SUB],
                                in_=x_sb[:, s0 + s, c * KCH:(c + 1) * KCH],
                                identity=id_sb,
                            )
                    xt = xtp.tile([KCH, TILE], mm_dt, tag="xt")
                    if xt_on_act:
                        nc.scalar.activation(out=xt, in_=tp, func=COPY)
                    else:
                        nc.vector.tensor_copy(xt, tp)
                    xt_sb.append(xt)

                # --- L1: h1 = lrelu(W1 @ xT)  [128, 512] ---
                h1_ps = psMM.tile([D1, TILE], f32, tag="mm")
                for c in range(NKC):
                    nc.tensor.matmul(h1_ps, lhsT=w1t_sb[:, c, :], rhs=xt_sb[c],
                                     start=(c == 0), stop=(c == NKC - 1))
                h1_sb = acts.tile([D1, TILE], mm_dt, tag="h1")
                nc.scalar.activation(out=h1_sb, in_=h1_ps, func=LRELU, alpha=0.01)

                # --- L2: h2 = lrelu(W2 @ h1)  [64, 512] ---
                h2_ps = psMM.tile([D2, TILE], f32, tag="mm")
                nc.tensor.matmul(h2_ps, lhsT=w2t_sb, rhs=h1_sb,
                                 start=True, stop=True)
                h2_sb = acts.tile([D2, TILE], mm_dt, tag="h2")
                nc.scalar.activation(out=h2_sb, in_=h2_ps, func=LRELU, alpha=0.01)

                # --- L3 folded: g3 = lrelu((d3 @ W3) @ h2)  [64, 512] ---
                g3_ps = psMM.tile([D2, TILE], f32, tag="mm")
                nc.tensor.matmul(g3_ps, lhsT=m3t_sb, rhs=h2_sb,
                                 start=True, stop=True)
                g3_sb = acts.tile([D2, TILE], mm_dt, tag="g3")
                nc.scalar.activation(out=g3_sb, in_=g3_ps, func=LRELU, alpha=0.01)

                # --- L4: g2 = lrelu(d2 @ g3)  [128, 512] ---
                g2_ps = psMM.tile([D1, TILE], f32, tag="mm")
                nc.tensor.matmul(g2_ps, lhsT=d2t_sb, rhs=g3_sb,
                                 start=True, stop=True)
                g2_sb = acts.tile([D1, TILE], last_dt, tag="g2")
                nc.scalar.activation(out=g2_sb, in_=g2_ps, func=LRELU, alpha=0.01)

                # --- L5: out = g2.T @ d1.T, batch-major via stationary swap.
                # Two matmuls into one 2-bank PSUM tile ([:, :392] in bank 0,
                # [:, 512:904] in bank 1), one strided ACT copy out. ---
                if no_l5:
                    nc.vector.tensor_copy(o_sb[:, s0:s0 + NSUB, :],
                                          x_sb[:, s0:s0 + NSUB, :])
                for s in range(NSUB) if not no_l5 else []:
                    g2c = g2_sb[:, s * SUB:(s + 1) * SUB]
                    if l5_single_bank:
                        poa = psO.tile([SUB, 512], f32, tag="po")
                        pob = psO.tile([SUB, 512], f32, tag="po")
                        nc.tensor.matmul(poa[:, :HALF], lhsT=g2c,
                                         rhs=d1t_sb[:, :HALF],
                                         start=True, stop=True)
                        nc.tensor.matmul(pob[:, :HALF], lhsT=g2c,
                                         rhs=d1t_sb[:, HALF:],
                                         start=True, stop=True)
                        eng_a = nc.vector.tensor_copy if (s in ocopy_dve) \
                            else (lambda o, i: nc.scalar.activation(
                                out=o, in_=i, func=COPY))
                        eng_a(o_sb[:, s0 + s, :HALF], poa[:, :HALF])
                        eng_b = nc.vector.tensor_copy if (s in ocopy_dve) \
                            else (lambda o, i: nc.scalar.activation(
                                out=o, in_=i, func=COPY))
                        eng_b(o_sb[:, s0 + s, HALF:], pob[:, :HALF])
                        continue
                    po = psO.tile([SUB, 1024], f32, tag="po")
                    nc.tensor.matmul(po[:, :HALF], lhsT=g2c, rhs=d1t_sb[:, :HALF],
                                     start=True, stop=True)
                    nc.tensor.matmul(po[:, 512:512 + HALF], lhsT=g2c,
                                     rhs=d1t_sb[:, HALF:], start=True, stop=True)
                    po_v = po.rearrange("p (b r) -> p b r", b=2)[:, :, :HALF]
                    o_v = o_sb[:, s0 + s, :].rearrange("p (b r) -> p b r", b=2)
                    if (split_ocopy and s % 2 == 1) or (s in ocopy_dve):
                        nc.vector.tensor_copy(o_v, po_v)
                    else:
                        nc.scalar.activation(out=o_v, in_=po_v, func=COPY)
               getattr(nc, out_dma_eng).dma_start(out=out_r[g], in_=o_sb)

    nc.finalize()
    return nc


def _np_dt(name):
    if name.startswith("float32"):
        return np.float32
    import ml_dtypes
    return np.dtype(getattr(ml_dtypes, name))


def _host_weights(W1, W2, W3, mm_dt_name=None, last_dt_name=None):
    mm_dt_name = mm_dt_name or BEST.get("mm_dt_name", "float32r")
    last_dt_name = last_dt_name or BEST.get("last_dt_name", "float32r")
    def pinv(W):
        u, s, vh = np.linalg.svd(W.astype(np.float64), full_matrices=False)
        return (vh.T * (1.0 / s)) @ u.T

    d1, d2, d3 = pinv(W1), pinv(W2), pinv(W3)
    f = np.float32
    mdt, ldt = _np_dt(mm_dt_name), _np_dt(last_dt_name)
    return {
        "w1t": np.ascontiguousarray(W1.T).astype(mdt),
        "w2t": np.ascontiguousarray(W2.T).astype(mdt),
        "m3t": np.ascontiguousarray(
            (d3 @ W3.astype(np.float64)).T.astype(f)).astype(mdt),
        "d2t": np.ascontiguousarray(d2.T.astype(f)).astype(mdt),
        "d1t": np.ascontiguousarray(d1.T.astype(f)).astype(ldt),
        "ident": np.eye(SUB, dtype=f),
    }


_NC_CACHE = {}

# Best configuration found via cost-model timeline simulation:
#  - r_xpose: f32r PE transposes run at full (2.4 GHz) rate vs half for f32
#  - bf16_out: halve output HBM traffic (rel err ~2e-3, well within 2e-2)
#  - ps_rebal: PSUM banks 3/3/1 (psT/psMM/psO) — psMM depth was binding
#  - out_dma_eng='gpsimd': output DMAs dispatch from the idle Pool queue so
#    input prefetch on the SP queue is never blocked behind output waits
#  - ocopy_dve=(3,): last L5 PSUM->SBUF copy on DVE to unload ACT
#  - dma_group=2: two 512-row tiles per DMA (3.2MB in / 1.6MB out) to
#    amortize per-DMA fixed cost (HW A/B: -4.9 us vs group=1)
BEST = dict(r_xpose=True, bf16_out=True, ps_rebal=True,
            out_dma_eng="gpsimd", ocopy_dve=(3,), dma_group=2)


def _get_nc(key="best"):
    if key not in _NC_CACHE:
        _NC_CACHE[key] = _build_nc(B_LOC, **BEST)
    return _NC_CACHE[key]


def kernel(x, W1, W2, W3):
    from concourse.bass_utils import run_bass_kernel_spmd

    x = np.ascontiguousarray(x, dtype=np.float32)
    w = _host_weights(np.asarray(W1), np.asarray(W2), np.asarray(W3))
    nc = _get_nc()
    in_maps = [
        {"x": x[i * B_LOC:(i + 1) * B_LOC], **w} for i in range(N_CORES)
    ]
    res = run_bass_kernel_spmd(nc, in_maps, core_ids=list(range(N_CORES)))
    out = np.concatenate([res.results[i]["out"] for i in range(N_CORES)], axis=0)
    return np.asarray(out, dtype=np.float32)



# revision 27
# speedup vs baseline: 1.0107x; 1.0107x over previous
"""Trainium2 Bass kernel for the 3-layer MLP encode/decode forward pass.

Computation (B = 65536):
    d_i = pinv(W_i)                       (host, negligible)
    h = lrelu(x @ W1.T)                   [B, 128]
    h = lrelu(h @ W2.T)                   [B, 64]
    h = h @ W3.T                          [B, 16]
    h = lrelu(h @ d3.T)                   [B, 64]   (folded: lrelu((d3@W3) @ h2))
    h = lrelu(h @ d2.T)                   [B, 128]
    out = h @ d1.T                        [B, 784]

Sharding: pure data-parallel — 8 cores x 8192 batch rows each; the tiny
weights (and host-side pinv) are replicated.

Per-core layout: activations are kept feature-major ([feat, batch]) so
TensorE contracts over features.  x is transposed on-chip via PE
transpose-mode in float32r (full 2.4 GHz rate; requires x/ident DECLARED
float32r — the BIR verifier rejects bitcasts of f32 DMA data as "not
rounded to FP32r").  The final layer swaps operand roles (stationary =
activation tile, moving = d1.T) so the output lands batch-major in PSUM —
no output transpose.  Matmuls run as float32r (full PE rate at
moving-N >= 256, ~tf32 rounding).  Output is written bf16 (halves output
HBM traffic; rel err ~1.7e-3 vs the 2e-2 gate) and upcast on host.

Schedule: PSUM banks split 3/3/1 (transpose staging / L1-L4 / L5) — psMM
depth is the binding pipeline resource.  Output DMAs dispatch from the
otherwise-idle Pool (gpsimd/SWDGE) queue so input prefetch on the SP
HWDGE queue is never blocked behind output sem-waits.  One L5 copy per
tile runs on DVE to unload ACT.

DMA: one 3.2MB transfer per 1024-row group in ([128, 8*784] f32, 8 batch
rows per partition), 1.6MB bf16 out — the in-group batch permutation
cancels between input transposes and output writeback.
"""

import numpy as np

B = 65536
N_CORES = 8
B_LOC = B // N_CORES  # 8192
D0, D1, D2, D3 = 784, 128, 64, 16
KCH = 112          # 784 = 7 * 112 contraction chunks for layer 1
NKC = D0 // KCH    # 7
TILE = 512         # moving free dim per matmul (one fp32 PSUM bank)
SUB = 128          # batch sub-tile (partition dim of x / out tiles)
NSUB = TILE // SUB  # 4
HALF = D0 // 2     # 392


def _build_nc(b_loc=B_LOC, mm_dt_name="float32r", last_dt_name="float32r",
              act_name="Lrelu", repeat=1, r_xpose=False, split_ocopy=False,
              bf16_out=False, xt_on_act=False, xin_bufs=4, outp_bufs=4,
              xtp_bufs=14, acts_bufs=2, out_dma_eng="sync", in_dma_eng="sync",
              alloc_mode="stack", staggered=False, no_l5=False,
              split_in=False, ps_rebal=False, hi_in=False,
              psT_bufs=None, psMM_bufs=None, psO_bufs=None,
              l5_single_bank=False, ocopy_dve=(), l5_bf16_psum=False,
              dma_group=1, body_repeat=1, split_out=False):
    import contextlib
    import concourse.tile as tile
    from concourse import bacc, mybir

    mm_dt = getattr(mybir.dt, mm_dt_name)
    last_dt = getattr(mybir.dt, last_dt_name)
    f32 = mybir.dt.float32
    LRELU = getattr(mybir.ActivationFunctionType, act_name)
    COPY = mybir.ActivationFunctionType.Copy

    nc = bacc.Bacc(trn_type="TRN2", target_bir_lowering=False, debug=False,
                   num_devices=N_CORES)

    xr_dt = mybir.dt.float32r if r_xpose else f32
    x = nc.declare_dram_parameter("x", [b_loc, D0], xr_dt, isOutput=False).ap()
    w1t = nc.declare_dram_parameter("w1t", [D0, D1], mm_dt, isOutput=False).ap()
    w2t = nc.declare_dram_parameter("w2t", [D1, D2], mm_dt, isOutput=False).ap()
    m3t = nc.declare_dram_parameter("m3t", [D2, D2], mm_dt, isOutput=False).ap()
    d2t = nc.declare_dram_parameter("d2t", [D2, D1], mm_dt, isOutput=False).ap()
    d1t = nc.declare_dram_parameter("d1t", [D1, D0], last_dt, isOutput=False).ap()
    ident = nc.declare_dram_parameter("ident", [SUB, SUB], xr_dt, isOutput=False).ap()
    out_dt = mybir.dt.bfloat16 if bf16_out else f32
    out = nc.declare_dram_parameter("out", [b_loc, D0], out_dt, isOutput=True).ap()

    n_tiles = b_loc // TILE
    G = dma_group
    n_groups = n_tiles // G
    GS = NSUB * G  # batch rows per partition in one DMA group
    # row = group*(512*G) + p*(4*G) + s  (4*G rows per partition -> one
    # 1.6*G MB DMA per group; the in-group batch permutation cancels
    # between input load and output writeback)
    x_r = x.rearrange("(n p s) f -> n p (s f)", p=SUB, s=GS)
    out_r = out.rearrange("(n p s) f -> n p (s f)", p=SUB, s=GS)

    with tile.TileContext(nc, num_cores=N_CORES, pool_alloc_mode=alloc_mode) as tc:
        with (
            tc.tile_pool(name="consts", bufs=1) as consts,
            tc.tile_pool(name="xin", bufs=xin_bufs) as xin,
            tc.tile_pool(name="xtp", bufs=xtp_bufs) as xtp,
            tc.tile_pool(name="acts", bufs=acts_bufs) as acts,
            tc.tile_pool(name="outp", bufs=outp_bufs) as outp,
            tc.tile_pool(name="psT", bufs=psT_bufs or (3 if ps_rebal else 2),
                         space="PSUM") as psT,
            tc.tile_pool(name="psMM", bufs=psMM_bufs or (3 if ps_rebal else 2),
                         space="PSUM") as psMM,
            tc.tile_pool(name="psO", bufs=psO_bufs or (1 if ps_rebal else 2),
                         space="PSUM") as psO,
        ):
            # --- constants ---
            w1t_sb = consts.tile([KCH, NKC, D1], mm_dt)
            nc.sync.dma_start(out=w1t_sb, in_=w1t.rearrange("(c p) m -> p c m", p=KCH))
            w2t_sb = consts.tile([D1, D2], mm_dt)
            nc.sync.dma_start(out=w2t_sb, in_=w2t)
            m3t_sb = consts.tile([D2, D2], mm_dt)
            nc.sync.dma_start(out=m3t_sb, in_=m3t)
            d2t_sb = consts.tile([D2, D1], mm_dt)
            nc.sync.dma_start(out=d2t_sb, in_=d2t)
            d1t_sb = consts.tile([D1, D0], last_dt)
            nc.sync.dma_start(out=d1t_sb, in_=d1t)
            id_sb = consts.tile([SUB, SUB], xr_dt)
            nc.sync.dma_start(out=id_sb, in_=ident)
            id_r = id_sb

            rep_ctx = (tc.For_i(0, repeat, 1, staggered_reset=staggered)
                       if repeat > 1 else contextlib.nullcontext())
            with rep_ctx:
             for _br in range(body_repeat):
              for g in range(n_groups):
               # --- load 512*G rows in one DMA: [128, 4*G, 784] ---
               x_sb = xin.tile([SUB, GS, D0], xr_dt, tag="x")
               if hi_in:
                   with tc.high_priority():
                       nc.sync.dma_start(out=x_sb, in_=x_r[g])
               elif split_in:
                   xr3 = x_r[g].rearrange("p (s f) -> p s f", s=GS)
                   nparts = 4 if split_in == 4 else 2
                   h = GS // nparts
                   eng2 = nc.scalar if split_in == "dual" else nc.sync
                   for pi in range(nparts):
                       eng = nc.sync if pi == 0 else eng2
                       eng.dma_start(out=x_sb[:, pi * h:(pi + 1) * h, :],
                                     in_=xr3[:, pi * h:(pi + 1) * h, :])
               elif in_dma_eng == "alt":
                   (nc.sync if g % 2 == 0 else nc.scalar).dma_start(
                       out=x_sb, in_=x_r[g])
               else:
                   getattr(nc, in_dma_eng).dma_start(out=x_sb, in_=x_r[g])
               o_sb = outp.tile([SUB, GS, D0], out_dt, tag="o")
               for t2 in range(G):
                s0 = t2 * NSUB

                # --- PE-transpose to feature-major: 7 chunks of [112, 512] ---
                xt_sb = []
                for c in range(NKC):
                    tp = psT.tile([KCH, TILE], f32, tag="psT")
                    for s in range(NSUB):
                        if r_xpose:
                            nc.tensor.transpose(
                                out=tp[:, s * SUB:(s + 1) * SUB]
                                    .bitcast(mybir.dt.float32r),
                                in_=x_sb[:, s0 + s, c * KCH:(c + 1) * KCH],
                                identity=id_r,
                            )
                        else:
                            nc.tensor.transpose(
                                out=tp[:, s * SUB:(s + 1) * SUB],
                                in_=x_sb[:, s0 + s, c * KCH:(c + 1) * KCH],
                                identity=id_sb,
                            )
                    xt = xtp.tile([KCH, TILE], mm_dt, tag="xt")
                    if xt_on_act:
                        nc.scalar.activation(out=xt, in_=tp, func=COPY)
                    else:
                        nc.vector.tensor_copy(xt, tp)
                    xt_sb.append(xt)

                # --- L1: h1 = lrelu(W1 @ xT)  [128, 512] ---
                h1_ps = psMM.tile([D1, TILE], f32, tag="mm")
                for c in range(NKC):
                    nc.tensor.matmul(h1_ps, lhsT=w1t_sb[:, c, :], rhs=xt_sb[c],
                                     start=(c == 0), stop=(c == NKC - 1))
                h1_sb = acts.tile([D1, TILE], mm_dt, tag="h1")
                nc.scalar.activation(out=h1_sb, in_=h1_ps, func=LRELU, alpha=0.01)

                # --- L2: h2 = lrelu(W2 @ h1)  [64, 512] ---
                h2_ps = psMM.tile([D2, TILE], f32, tag="mm")
                nc.tensor.matmul(h2_ps, lhsT=w2t_sb, rhs=h1_sb,
                                 start=True, stop=True)
                h2_sb = acts.tile([D2, TILE], mm_dt, tag="h2")
                nc.scalar.activation(out=h2_sb, in_=h2_ps, func=LRELU, alpha=0.01)

                # --- L3 folded: g3 = lrelu((d3 @ W3) @ h2)  [64, 512] ---
                g3_ps = psMM.tile([D2, TILE], f32, tag="mm")
                nc.tensor.matmul(g3_ps, lhsT=m3t_sb, rhs=h2_sb,
                                 start=True, stop=True)
                g3_sb = acts.tile([D2, TILE], mm_dt, tag="g3")
                nc.scalar.activation(out=g3_sb, in_=g3_ps, func=LRELU, alpha=0.01)

                # --- L4: g2 = lrelu(d2 @ g3)  [128, 512] ---
                g2_ps = psMM.tile([D1, TILE], f32, tag="mm")
                nc.tensor.matmul(g2_ps, lhsT=d2t_sb, rhs=g3_sb,
                                 start=True, stop=True)
                g2_sb = acts.tile([D1, TILE], last_dt, tag="g2")
                nc.scalar.activation(out=g2_sb, in_=g2_ps, func=LRELU, alpha=0.01)

                # --- L5: out = g2.T @ d1.T, batch-major via stationary swap.
                # Two matmuls into one 2-bank PSUM tile ([:, :392] in bank 0,
                # [:, 512:904] in bank 1), one strided ACT copy out. ---
                if no_l5:
                    nc.vector.tensor_copy(o_sb[:, s0:s0 + NSUB, :],
                                          x_sb[:, s0:s0 + NSUB, :])
                for s in range(NSUB) if not no_l5 else []:
                    g2c = g2_sb[:, s * SUB:(s + 1) * SUB]
                    if l5_single_bank:
                        poa = psO.tile([SUB, 512], f32, tag="po")
                        pob = psO.tile([SUB, 512], f32, tag="po")
                        nc.tensor.matmul(poa[:, :HALF], lhsT=g2c,
                                         rhs=d1t_sb[:, :HALF],
                                         start=True, stop=True)
                        nc.tensor.matmul(pob[:, :HALF], lhsT=g2c,
                                         rhs=d1t_sb[:, HALF:],
                                         start=True, stop=True)
                        eng_a = nc.vector.tensor_copy if (s in ocopy_dve) \
                            else (lambda o, i: nc.scalar.activation(
                                out=o, in_=i, func=COPY))
                        eng_a(o_sb[:, s0 + s, :HALF], poa[:, :HALF])
                        eng_b = nc.vector.tensor_copy if (s in ocopy_dve) \
                            else (lambda o, i: nc.scalar.activation(
                                out=o, in_=i, func=COPY))
                        eng_b(o_sb[:, s0 + s, HALF:], pob[:, :HALF])
                        continue
                    po = psO.tile([SUB, 1024], f32, tag="po")
                    nc.tensor.matmul(po[:, :HALF], lhsT=g2c, rhs=d1t_sb[:, :HALF],
                                     start=True, stop=True)
                    nc.tensor.matmul(po[:, 512:512 + HALF], lhsT=g2c,
                                     rhs=d1t_sb[:, HALF:], start=True, stop=True)
                    po_v = po.rearrange("p (b r) -> p b r", b=2)[:, :, :HALF]
                    o_v = o_sb[:, s0 + s, :].rearrange("p (b r) -> p b r", b=2)
                    if (split_ocopy and s % 2 == 1) or (s in ocopy_dve):
                        nc.vector.tensor_copy(o_v, po_v)
                    else:
                        nc.scalar.activation(out=o_v, in_=po_v, func=COPY)
               if split_out:
                   or3 = out_r[g].rearrange("p (s f) -> p s f", s=GS)
                   h2 = GS // 2
                   getattr(nc, out_dma_eng).dma_start(
                       out=or3[:, 0:h2, :], in_=o_sb[:, 0:h2, :])
                   getattr(nc, out_dma_eng).dma_start(
                       out=or3[:, h2:GS, :], in_=o_sb[:, h2:GS, :])
               else:
                   getattr(nc, out_dma_eng).dma_start(out=out_r[g], in_=o_sb)

    nc.finalize()
    return nc


def _np_dt(name):
    if name.startswith("float32"):
        return np.float32
    import ml_dtypes
    return np.dtype(getattr(ml_dtypes, name))


def _host_weights(W1, W2, W3, mm_dt_name=None, last_dt_name=None):
    mm_dt_name = mm_dt_name or BEST.get("mm_dt_name", "float32r")
    last_dt_name = last_dt_name or BEST.get("last_dt_name", "float32r")
    def pinv(W):
        u, s, vh = np.linalg.svd(W.astype(np.float64), full_matrices=False)
        return (vh.T * (1.0 / s)) @ u.T

    d1, d2, d3 = pinv(W1), pinv(W2), pinv(W3)
    f = np.float32
    mdt, ldt = _np_dt(mm_dt_name), _np_dt(last_dt_name)
    return {
        "w1t": np.ascontiguousarray(W1.T).astype(mdt),
        "w2t": np.ascontiguousarray(W2.T).astype(mdt),
        "m3t": np.ascontiguousarray(
            (d3 @ W3.astype(np.float64)).T.astype(f)).astype(mdt),
        "d2t": np.ascontiguousarray(d2.T.astype(f)).astype(mdt),
        "d1t": np.ascontiguousarray(d1.T.astype(f)).astype(ldt),
        "ident": np.eye(SUB, dtype=f),
    }


_NC_CACHE = {}

# Best configuration found via cost-model timeline simulation:
#  - r_xpose: f32r PE transposes run at full (2.4 GHz) rate vs half for f32
#  - bf16_out: halve output HBM traffic (rel err ~2e-3, well within 2e-2)
#  - ps_rebal: PSUM banks 3/3/1 (psT/psMM/psO) — psMM depth was binding
#  - out_dma_eng='gpsimd': output DMAs dispatch from the idle Pool queue so
#    input prefetch on the SP queue is never blocked behind output waits
#  - ocopy_dve=(3,): last L5 PSUM->SBUF copy on DVE to unload ACT
#  - dma_group=2: two 512-row tiles per output DMA (1.6MB bf16) to
#    amortize per-DMA fixed cost (HW A/B: -4.9 us vs group=1)
#  - split_in: input of each group as two 1.6MB DMAs so transposes start
#    after the first half lands (HW A/B: -6.1 us vs one 3.2MB DMA;
#    quarter-splits and split outputs both regressed)
BEST = dict(r_xpose=True, bf16_out=True, ps_rebal=True,
            out_dma_eng="gpsimd", ocopy_dve=(3,), dma_group=2,
            split_in=True)


def _get_nc(key="best"):
    if key not in _NC_CACHE:
        _NC_CACHE[key] = _build_nc(B_LOC, **BEST)
    return _NC_CACHE[key]


def kernel(x, W1, W2, W3):
    from concourse.bass_utils import run_bass_kernel_spmd

    x = np.ascontiguousarray(x, dtype=np.float32)
    w = _host_weights(np.asarray(W1), np.asarray(W2), np.asarray(W3))
    nc = _get_nc()
    in_maps = [
        {"x": x[i * B_LOC:(i + 1) * B_LOC], **w} for i in range(N_CORES)
    ]
    res = run_bass_kernel_spmd(nc, in_maps, core_ids=list(range(N_CORES)))
    out = np.concatenate([res.results[i]["out"] for i in range(N_CORES)], axis=0)
    return np.asarray(out, dtype=np.float32)

